# revision 1
# baseline (speedup 1.0000x reference)
# Self-contained Trainium2 kernel for nn_CLOPLayer_74586402062828.
#
# The reference applies a fixed permutation (CLOP conditional neighbor swaps,
# jax PRNG key 42, p=0.9 -- fully deterministic) along the flattened H*W axis
# of x[512, 9, 84, 84].  The permutation is a compile-time constant, hardcoded
# below (zlib+base64, int16 little-endian, 7056 entries, crc32 3102470381).
#
# Strategy: data-parallel over batch across 8 NeuronCores (64 batches each,
# 576 rows of 7056 fp32 per core).  The gather runs on GPSIMD ap_gather whose
# cost is ~(num_idxs/4)*(129+5d) Q7 cycles, so d (rows per index) is maxed at
# 18: 576 rows = 2 sets x (16 partitions x 18 interleaved slots).  Positions
# are split into 16 slices of 441 over 2 rounds; each Q7 16-partition group
# gathers one slice per round from a tight window of the input (displacements
# bounded in [-504, 422], window <= 947).  The host pre-builds the
# interleaved windowed layout and decodes the gather-output layout, so the
# device does only: staircase DMA-in -> ap_gather x8 -> DMA-out, with
# I_A/I_B ping-pong across rounds overlapping DMA and gather.  Input loads
# issue on the SP queue family, output stores on the Activation family, so
# stores never queue behind the next round's prefetch.

import base64
import zlib

import numpy as np

_PERM_B64 = (
    "eNoNlwO74DoABZsaaZMmVdq1bfuubdu2fRd3bdu2bdu2bXv3vf9wvpk5HFeV4zmZE7manMTV5nSuLqdxkGvImZzFIa4pt5OjnMMR"
    "zuVacz4XcIwLuYjryMXnEnBduO5cIi4p14NLxp3henEpuFRcP64/N4BLy6XjBnOZuDhuCJeFG8YN57JzObkc3CguN5eHG8fl4yZw"
    "+blCXEFuKleYe8UV5WZwJbiS3CyuFFeGK83N58pz5bgKXGXuO1eJq8JV41ZwK7kanMDV4upwCqdyEmjAbeQMrjG3ldvGYc7mdnC7"
    "uFacx7Xk2nD7uHZce+4gF4/rxHXmEnJducRcN643l4RLzqXk+nLnuNRcGi6Wu8Kl5zJwg7iM3FAuM5eVy8aN5EZwd7ls4CE3lhvP"
    "5eUmcpO5AlwM94Irwk3hpnHFuWJcEVAczObmcGW5T9wCrjT4xi3hfnA/ud/ccg5wfzkAVnPruLWcCHhQj5NBfW4L14jbzDXhmnEQ"
    "NOcQaAJssJcjoC13iNvPBeAId5g7yh0GJ7jjXAKQEJziTnPJQRLQh7vEpQAXuL5gIJce3OCucde521wGcIu7ww0BWcFobgz3gHvM"
    "5QaPuCfcU+45N4krAF5yr7k33FtuJveBe8/N5eZxJcEcsIhbyC3mKnLLuIWgAvjDVedWcRz4x63hBLCe28DVAnWBBuoBHTQAm8F2"
    "rhFowWGwh2sBXOCAA1xr4IMOXAgYaA+OcZ3ASa4nlwgkBklBD9ALJAMpwUUuNbjKpQGXuQEgLbjJDQaZwFBwj8vF3eeygxwgJxgN"
    "nnFjwTiQH0wA07mCoBCIAYXBO+4jVxQUA5+5EuAL95UrA8qCcuAXtwiU45eAKqAy+A2qgepgFagJagMFrAUq2MTVBwZoCDR+K2gK"
    "dnMUNAPNQUuwB3igDWgL2oEIxAMdQXzQBRwBXcFJcJZLyJ/neoJzoA9IBS6CfqA/SAcGgYEgI7gOMoMsYBjIwmfiR4JRYAzIA3KB"
    "vGA8yMVPApPBS/AcTAEvwHTwFrwGM8FsUArMBfPAZ7AAlAdLuYrgJ/gBfoGqYCWozFflV4M1gOcFfh3YAGRe4S1ggm2gMdgOdoId"
    "YBfAfCuwFxD+ANgPOoCDwOePgs7gMH8cdAMnQCK+OzgDkvBnwXlwAaTiL4FYkIa/Aq6BOHAT3AC3wHAwAtwHd0BW/gHIzj8Ej0E+"
    "MBE8BXn5qWAaeAXegRngDZgF3oMP4BP4COaDL+Ab+AoqgcVgGVgK/oDloAb4B1YAwNcBq3iJr8lvBLV5ld8C6vMN+Ia8ySPe4pvw"
    "Tfnm/D6wj6e8y7flD4GQb88fA/H5ePwxvgt/CiTme4NkfE++F3+Wvwz68uf5tPxVMIBPz1/lM/KZ+dvgLrgHhvHD+Rx8Tj43/wTk"
    "4cfzz0A+vgBfkJ/Ex/CF+SJ8Ub4Y/5YvzpfkS/Gl+TL8dzCfL89X4Cvylfi/oArP8dX4lfx6UIevxYv8JlCXr8frvMFDvhHfmN8N"
    "mvE27/At+FZ8S74134Zvxwc84w/xEX+E78yfBsf5bnx3vgeflE/BJ+dT8n341Pwlvj+fjs/AD+Sv8XH8DX4on40fwY/kH4Ex/Ch+"
    "ND+WH8fn5yfwz/mn/GQ+vzCVn8bP4KfzJfj3/Gz+Az+X/8TP47/wC/nF/CJ+Gb+U/8Uv5ysJ1XlOWM2v5dfw6/gN/HpeFjbxm/kt"
    "vC5s56FgCDv53TwS9vAev5ffz3vCQb4DHwgJ+E78Uf4Ef1Q4yScUEgu9+TN8EqEfn0y4yF/mB/Gx/GB+CJ9euM7f5DMJt/l7/FDh"
    "AX+XzyY85HMKj/jcwkPhGZ9PmCRM4Z8KhYQ3/Ex+Fv+On8N/5D/zZflSwlf+G/+dX8L/4H/yFYUV/F++Bv+PBwIvCIIk1BI28opQ"
    "R6gnbOXrC6awg7eExgIWbMERiEAFV2gt7BPaCkxoLzDxkNCVjy+c4k/ziYSkwjk+hXCBTy70EVIJ/YQ0whU+rZBOGCQMFjIKd/hh"
    "QhbhPp9DyC6MER7zT/iJ/DhhvPCCLyC84mOEwsJrvphQVCgiFBdKCLOFBfwnoaxQWlgolBN+8xWEP3xloYpQTVgu1BBWCqLAiTWF"
    "uoImqEIjYRvfQNgs7OJ1sYnQTGgu7BJ2Cwf4VoIvdOQPCAeFeEIkJBCOCZ2ELkI8sYfQXegl9BR6CymFvsIF4aIwQEglDhQyCLf4"
    "60JmYYiQVRgujBRGCKOEXEIeYbSQV3gsTBBe8gWFKcJk4aUwQ3glvBZmCYXFksJcoZhYRlggfBbKC4uFpcIiYZmwRFghVBX+CKuE"
    "1YIg1hbWCeuFDcJGYZNQR9wibBeaCtuEHUILwRRbCnuENgIR9wvthFDoIHQUjgihGInHhW7CKeGkcFpIIp4RzgvnhGRiCvGSECuk"
    "FtOIacVrwg0hg3hTyCjeFW4LWcT7wj0hpzhWyC1OFJ4Iz4Tx4gthmjBVmC7EiG+Fd8IHoag4T5gvlBS/Cl+E78I3oZL4Q/gp/BN+"
    "C3+F6gIQq4m8uFJcLYqiLCpiQ6GuWF80xIbiTqGx2ERsJtoiFR1xr9BKbCu2EQ8LvthZSCh2FTqJCcQTQmKxq3hWSC52F1ML/YXL"
    "wkWxr9hPvCqkF9OJt4RM4h0hThwm3hSHi9nER0J2sRCfS8wr5hGfC/nFgmIhsYD4RigizhSKi2/EEuJHYY4wRywjlhLLiV/E8uIv"
    "oYK4TKwoVhHXCCvEVeJaQRJriYJUW1RFTdwqNBA3iY1EJFriTnG72EJsLu4SXdETA7G12F5sJ3YQO4rxxc7iMfGE2FM8KXYTe4m9"
    "xR7iOTGleEWIFeOEAeJlcZA4REwnZRazig+E2+IIcaQ4WhwljhHHivnEieI4cYI4RZwsThInSNPFqeJ7Ybb4TiwtzhI/iXPF+WJZ"
    "8aM0T1okLhEri0vF5WJVsbr4T6wprhHXiuvFemJNaaO4RYRiPWmbuFXEYlNxt7hV2iO2FPeJe8X94gHxoHhI3CcdFY+IXcTjYiIx"
    "oZRUPC2eFc+ISaTzYnIphXRJvCIOFK+JV8XBYnppqJhRipPuiHfFrFJ2Kad0X3woPhafiU/Eh9IL8bn4UnwlvhYLSW/FmeIH8b34"
    "UZwnlpAWiAvFxeIP8af4Xfwl/hF/i3/FGiInVZeAtEqqJW0QZWm9pEqaZEgNJSjtEE3JkppKRLKlZlJLyZFcyZcCqZ0USpHUSYon"
    "JZC6SImkxNIFsY+YVDot9RfTSH2kvlJ/Ka10Q7wu3hIzSDnEIVJm6Z74QBwhPRJHSrmksVJu6amYX5omFpTySTPEF1JhqYhUVJop"
    "zZA+iyWlUtI38atYVloglZMqSEulilIVaZlUVVou8dI6UZJqS4pURxKlzWJ9SZcaSI0kJDWRsNRcaiFRaZfUWiJyG6mVtF9qL3WQ"
    "OkoHpc7SKbGjHF/uLvWQekrJpN5SSqmflEpKIV+SBkgDpXTyVSmTlEUaLt2Whkl3pBxSNimbPFrKI+WVxkuTpIlSAWmyNEV6KU2X"
    "XknFpXfSbGmOVEyeJZWRvkrlpUXSQum7VElaIq2QKkvVpJVSDemvtEZaK62TJHmjVFfaJDWWtkh1ZVPeIUF5t7RT8iRbbisdkhz5"
    "gOTLh6X4EpOPSsekrtJJqZt0Skoi95LOSonl89JFKbUUK12WUsuDpMFSWjmDfFPKKN+SRkl3pfvSGOmxlF0eJ+WSc8vPpKfScylG"
    "mipNk95IheRi0lvpvVRamil/kkrIpeQv0jdpsVRO/iFVlH9JfyROXi39k6rJ/wtKXi1vkGRZkXV5s6TJhrxd2iZZchN5j9RU3itR"
    "+bDoykzy5LZyO7m9fEiO5BNSQjmBfEZKJJ+TksqJlZ5ySjm5nEq+Il2T0sg3pOvSUCm9PFgeImeRh8m35RFyTvmRdE9+IuWVx8nj"
    "5YJyfnmsMlmOkafJReUPUhF5rlRcni99lsrI8+Wy8k9pofxbqiBXkivLVeUq8j+5uvxHrinXlmvJqlxHri/XkxvIjeTG8lYZy0hu"
    "ITeTm8st5T1ya3m/HMgH5FA+LsWTO8lH5eNyV7mT0k0+JSeTL0in5b5yb7m/3E+OlS/KA+VBcmY5kzxUjpPvSVnlB9Id+YGcQ74v"
    "j5XzyPnkCfIzuYD8WpoiF5any1PlGfI7uaQ8W54jf5RLy5/l8vICeZG8TF4sL5GXykCuIS+XV8pr5HWy+H/4rZcVZYO8Wd4ib5Ib"
    "ytvkHfJOebe8Xd4lt5KJ0kbeJ3vKQbmDfEQ+LHeWj8kn5S5yAiWh0ktOovSRz8u9lXPyZfmSnEq5IqdRrskZlAHKDTmjMly+K4+U"
    "R8mj5THyYzmH8kjOrUyUJ8njlZdyXqWAUlCJUQor05SiSjGluFJC+SZ/kr/IZZTvcjnlh/xbrqCskCsqf+W1MqdUV2opoiIoG/8f"
    "QD2lrmIoutJIgYqtNFGQgpW9MlUcxVWOSG2UdgpTDirxlA5KfKWH3F1OpJyRkyrJlO7KBTm5klIZIKdWrspplevyTfmWnEnJrKRX"
    "hirZlKxKduWhPErJqYxR8ijP5RdyPqWQ8kp+Lb+RXyrv5VnyXPmDPE+erZRWSilf5Z/yQqWs8kuupFRWVslVlSpKNeWvslpZpUhK"
    "NbWOoiqaUl+xlAZKQ2WbYipNleZKM6WF0lLZq7RSfKWtckA5rERKqHRWTshdlK5KN+Ws3FM5o/RSUijnlPNKXyVW6a+kU1Krg5U0"
    "6iDlupJFGaaMUO4qI5XRSi7lifxUnqCMU/IrE5VnymTlhfJWLqJMV2YoM5VZynulpDJXKaHOVT8r5ZXFyiLlj7JEKa8uU34rQKmh"
    "/FOAWltZr2xQeHWTslnRVFVtrDRQdyjblcbqHgWpuxVb3ae4anslUA4pR5WOSqgeUyI1oXpSSaCeUk4rSdQ+Sj8lpXpRuaykUi8p"
    "adUhyk0lTkmv3lJuK8OVO8p95Z7yQMmuPlIeK7nVp0oe9bkyRZmqFFRj1NfKW2WqOkf5oMxTPikl1QXKF+W78lX5qSxXKqi/lBVK"
    "ZZVTayprlTXKOkVU16iyqqgblS1KfXWr0lA11Z0KVqm6S2mtEHW/4qit1UClGlPjqUeUE0onNb6aSO2hnFWSqonVZGpytbd6QRmo"
    "XFGuKrHqADWDekPJqGZWM6lZ1GHqCDWb+lAZpT5Rcqp51XFqPjW/WkCdor5RnquF1XdKUXWa+lEprs5W5yvflDJqWXWpslD9oSxS"
    "K6krleXqMlVQV6o11FWqpNZW66jr/xeVoerqFlXVLLWJ2kxtqjbRmqst1Vaqp/pqW/WA2kFtrx5VjytH1GNqF7Wb2l09pfZSe6pn"
    "1BRqHzWZ1l+9pqRTB6qD1MFqnHpDHaoOV7OqOdSR6j31oZpLHatOUh6rebQJ6iT1lVJILaLOUKerxdRZ6kx1jlpKLa3OUz+pC9Sy"
    "Wjl1sfpD/alWUauq1dUV6mq1prpOXavWUgWtrrpR3axCtZGqa1vV7eoOtYW6W92j7lL3qvvVNmo79ZB6UO2oHlaZ1lU9rp5Q42un"
    "1R7qWbWvmlRLrp1TL6qX1CvqZfWaelUdombQbqp31IzabfWuOlrNomXTxqijtPFqbu2ZOlmdqL5QJ2ov1TdqYe2dOk2brr3VPqgf"
    "1fnqHK209k1dolZUf6lL1YpaZe2vymnLtCpaNU3U6qmytkHdpNbV6mnbVEODWiPN1CytmYa15to+1dE8zdUCLdR8rbPaUYunJdBO"
    "qgm1RFo37byaWLug9lNTaBe0lFoq7ZKWRkunDdLSa4O1TNotNat2W7uv5tDuaI/UXNoT9ak6TiugvVLzaQW112oh7a0aoxXVSmjF"
    "tZLaZ/WrOk+bq31X52vltYXaIm2J9kP7pwJthcZrNbXVWm1tjVZHUzRN26ht0hpqO9WtmqHb2g6NaDu1llorrY22R2unOXp7LdI6"
    "aEe0UO+ixdOTaN21HloivZfWW+uj9dVSa/21tNoA7bo6UIvTMmsZ9RvaUG2Ell17oN7TcmpjtZx6Xm28ll97qk3WJuhTtSnae/W1"
    "NkObqRXTZmvF9WJ6GW2BVk77plXQfqt/1KXacq2ivkqrrtXQgF5L4/W12nptg1Zfa6Bt0RprmzWkNdW2a7u0FlpTfa/WWtunufpB"
    "ra12WDukddY6ace0E1pX7aR2SjujJdbPaUn081o/LVa7qF3RLmvXtLR6en2INkzLoF/XM+sjtdHaMP2B9lAboz3WJmhPtElafv2Z"
    "9lJ7pRXU32jvtFlaKe299kX9qJXQF2ul9TL6d+2nVkn7pVXV/mp/tJXaP43T12mrdEmv8f+dVvU6uqZv03Qd6qaOdEvfrVHd1pvr"
    "+7UDmq97OtPb6+30DvpxLYEeX0+on9a66Un1Xnoy/ayWUu+tp9JT61e1NPp1rb9xUxusD9du6ln07Pp9LZs+Qr+r59Dz6KP151pe"
    "/YX2XC+gF9Fi9Nd6Eb2o/kErqX/WvmiftFL6V62svlAvpy/Sf2uL9cp6FX2ZXlVfrq/QRV3Wa+mKvl6vp9fXN+rb9EZ6Y32rjnWi"
    "Q6OF3sRordvGPr2tflQ7oHfWI72jfkLvpHfVe2on9e56T/2Mfl7vo/fV++v99JRGOn2gPki/pd3VMulD9Fv6bf2e/kjLpY/RH+r3"
    "9XH6I32iPl6fpE/WX+hT9cL6K/2NPkufrc/U5+hz9Xn6fP2zXkFfqi/Qf+g/9ZX6b726LuicUVNfowOjrr5Ol4wNegO9oa4ZhtFE"
    "N40d+k59l95K36tTo43e0nCNw/p+/Yh+SD+mH9W76PGNHnpCI5GRXE+hn9WTGcmNFEasfkUfoF/TUxtx+lD9hp5VH67f0TMbWY1s"
    "xlg9t55Pz2k80Z/q+YxC+hT9pT5Nn6G/1YsahY33+jv9o/5JL2mU17/q3/QyRiV9ib7IqKZXMiobq/V/ejVDNGoYa3XF2KTLhmps"
    "1rfoDYzt+majsYGM3Xpzo5lBDM8I9FaGbzBjv9HeiGdERieji3FK72ac1pMYSY0L+jn9kn5R72ukMmKNq3o6I72RwRhoZDEyGSP1"
    "UfoDfbiR3chlPNZzG8/0cUZeY5JR0ChgTDEKGdP1D3oRY4Yx05htfNHnGqWNecZ3fYFR3vilVzT+6H/1KkZVY6VRWxeMmkZto45R"
    "16hn1DLqG1uMTUYzXYNNDWzs0VvqLQzHwHCv0c7YZ7SBodHR6GAc1zsbx4zERlejh9Hd6GX0NHobfYzLeneYxhhkpDWuGION60ac"
    "cdMYatwxbhk5jJHGKOOBMdrIDh8Zj41ccKIx2YgxJhhTjWnGdKOY8c4oYcwy5hjzjVJGWaOcsdBYbHw1ysElxm/jl1HdqAxXGWsM"
    "Dq4zBLje2GDoxkajodHI2GFYhgER3GnsMlobe4yDelujjeHCwDhotIOHjaNGAuO40RF2gieNRPC0kRieM/oZF4xLxkXjsjHAuGpk"
    "NK4ZN4whRgY4wrhtZIH3jKzwvvHQGGuMN54a+Y288IXx3HhpvDIKwTdGceOt8cF4b3w0Phkl4Bfjm/HdqGD8MH4ay4zlRiW4wlgG"
    "q8LqsBpca4hwDawDZajCenCr0QBuMxpCC5rQhrsNCgk8YDhwLzxkBPCIEcIIdoYnjFNGQnjGSALPGklhMpgcpoSpYAqYGvaHqc20"
    "cBDMCIcZmWBmmAPeNcYY2WAe4x7MDZ8YeeAzIx98Al8b+eEk+Bq+hFNhMVgUFoefjZKwDJwDS8OysDxcaiyCf4y/RkVYBa42AORh"
    "DSjBmlCBglkX6rA+3AIbwc1wK2wKt8MWsBlsBZvD3dCD+2Bb2B4yeBDGhwlgPNgFdoPxzRPwvNEL9oRnYD94HvaFF2EsTAcvw4Ew"
    "Dg6GQ+BwOAwOhSNhTjgCjoaP4Bg4Fo6HBeBEOBnGwMLwGZwGp8PC5hs4E5aCs+Fc+BHOhwvgQlgBLoZL4RL4z/gDV8IV8B+sBaua"
    "teE6uB5uhBsghIq53ahn6mYTuAPugjthS7gH+pCYjnkAumYHeATGMw/Do7ArPAZ7wFMwkXkWJjV7wz4wmXkJpoED4BWYHqY1b8Dr"
    "8Ca8BW/Du/AOHAWzmg9gDnMcnABzmc/hUzgFvoAF4Ss4A8aY72Ehs4hZ1PwES5klzM/wK5xnfoc/4Q9Y3lwOf8PK5iq4GgKTN9fC"
    "GqZs1jZVcxPUzG2wMTTNhmZj0zYts6lpWdRsZe6Hh6BnBmY78zhkZmR2MhOYic2T8DTsbp6DF2AKM7mZ0kxl9jP7m2nMWDO9mcEc"
    "bGYyM5tZzOFmdvMhvA9HmrnNxzCfmcfMa+Y3C5gFzSLwLZwF38EPsJhZ3JwHZ5tf4DdY1vxsljZ/wQpmJfOb+RdWMTlztVnNXGnW"
    "NEVTMuuadUzD3Gg2MDebjUxkNjF3mK1hc7OF2dL0zdZmG7OtGZodzPZmR/OYmdDsYh41u5knzJ5mD/OM2dfsbfYxr8JL5kVzgJna"
    "GmQOsOLMoeYw87Z5y7xjjjJzmvfNnNYDc7w51pxoTjAnma/MyeY0c7o51ZxlzjTfmO/NuWYZc45ZzlxgfjfnWhXNJeYK85e53Fxm"
    "rjKrWSusWuZac725ztxg1jc3mfD/mNpmbjUbWc1MbO42d5nY2mvuMw+Y+82D5iEzsI6Yh6zI6mqeNJOYp80uVjfrvHnWvGCeM6/B"
    "y+ZAM52Z0UxjXTNvmjfMIWZGK5N117xnjrBGm+PMR2Zu64n5zHxqPjenmC/MgtYM86352ixilTSLWR/MT2YxVNJaaC4yy1gLrJ9m"
    "Beu3+cdcai2z/plrTMHirdWWaKmWZNWxtpj1LcOClmnZ1k4TWcRqbjlWC8u1Wlm+xawDVmezgxXP6mR1tk6ZvczEVg8rqdXbSm6d"
    "sc5bKa0rZqzV37pqDrQyWxmsbOYIc4iV1RpjZrcemo/NXFYOK4+Vz8prTbTyWy/NGKuQ9c4sbE23PprFrRLWfLOU9dUsa5VE5a0f"
    "5lKzklXRqmxxVnUTWNWtVVZtS7ZqWuusepZi6VYDq6FVD22zmlpNrD3mDqulRS3Pouiw2dpqa4XWcfOI1dFKaMW3ElkJUFeru9XT"
    "SmGdtVJZfa1+1iXrupnOumJdtQZbcdZQKyMaZmWzRlp3rNHWKGuMNdYqYI2zxlsTrEnWc+uF9dKaZhW1ZlhvrffWbOuLOc8qbZWz"
    "FlqLzUVWWbTEqmL9NZdbVa2VVg2rPAaIR3Wt9dYGa6O1ydpsbTcNtN3aalmomYXQLmuPtddqY7movdXOOmglsA5bITpqHbOSWCes"
    "U1YvKz7uYyVB56wUKK112UqJ0luDrGvWDeu6dcvKYt22hlv3rCzogfXQyoEeWU+sx9ZT65k12ZpqvbamWDFoMpppzbLeWXOs4mgW"
    "mm99sb5ZpVA5VB4ttn5bv6zK6K/1z6plVUNrLREJSEa1UF2koC1WY6s+aogaoZ2WjZogjAhqgfZbrVEr1Ab5iKHjVoTio2PopNUZ"
    "dUWJ0EnUA/VEF6xeqC+6aKVGaVAsSo8GoJtWZpQB3bWGovtWVpQN5UTZ0Wg0FuVFeVA+VAC9svKjZ+gFKoKmounok/XB+mh9tkqj"
    "r9Z364dVBlVAP63F6I/FoaqoElqJaqA1loRqotpIRXWQhnQEkYkaoMaoGWqKdlvN0T6rJXKQh9qiAB1E7VE71BHFQ51QQnTaSooS"
    "o2RWMnQWJUepUFLcD/VHl9BllA4NRINRHLqOMqFhaDjKhu+gXGgUeoDGoPFoAnqCCqGJqCB6YxVGRdEMNA3NRO/QbDQHzUXz0Hy0"
    "AC1Ei1BFtBQtQVXQCvQbVUer0Gq0Bq1DG5CA16MtaBNS8Ta0EW1HW9EOtAvtRLvRXrQHUbwPdUAH0GF0CB1FR9Bx1AV1Q6dRd3QK"
    "dcdJ8DmUDPfGF1FadAUNQlfRNZQOD0EZ8C00At1GI9FddB/lRjlwbjwOPUaT0HP0FE1BE/Br9BK9RTG4BCqOi+I5+AMqgT+hr+gb"
    "+o7K4p9oOfqF/qBl+B+qjjm8FtXANXEtXBvXxZvRRqxhiBtiEzfGpo1wU0ywg128H3nYxwwHOMQRjoc74wT4DOqKE+HeqBs+j3rh"
    "FPgC6oP74f74BhqAB+L0OCPOhDPje2gozoqz44doFM6Fc+I8OC/Oh/PjgvgVeoMK4Wn4PXqNi+GP6DP6gkrjUrgcLoN/oIV4GaqI"
    "l+Aq+C+qigGuhnksYgnLWMHrcD2s4wZ4E65nW7gJ3oEx3ol34Ra4FW6DW+O2eD/ugDvi9vgIPoES4sQ4nn0a98R9UHLcF6fCKXFq"
    "fBGnxbH4JrqKB+NBeAQegrPgYXgkHo5H40doDB6Lx+HxeCIugCfhZ/glLoyL4Kl4Bp6JZ+GS+AOei+fh+biU/Q1XwJXwd1wZL8Ur"
    "8G+8Cv/Bq/EavBYL9hpbtDfj+tjAW/BWbNjNsI2bY2y3xLZN7Bb2btuz22Hf7oQP4cgO7eP4JO6Ce+BT+Cw+gy/gcziFnQZfwpfx"
    "FZzGjsMD7Ov4Br6Dr9v38G2c1X6As9lP8EP8GOeyn+LneAqejCfar/AbHGNPs9/i93g2/og/4c94rr0Al7F/4MW4vF3B/ouX48o2"
    "Z//DvF0HA3sDXo/r2LKt2o1wfbuBvR1bdhMb2rvxHrwP78WO7dqt7Hb2YdzWPoqP4fj2CdzFTmh3trvZ3e2edjL7PE5pp7KT2/3s"
    "azitncG+iW/h9HYmO7M91M5i37az2yPt0XZOO7c9xh5v57fz2ZPsF7iQPR2/w0XtwnYxu7g9w/6CS9qL8Fdczp5vL7TL2pXsivYy"
    "e7ld1a5hr7Sr25K92q5tK3ZdW7Nlsg03tBvZje1t9na7mU3tA7ilvdM+iNvYzA7s9nYH+7DdyU5gRySxfcJOYnchSe1edh/7nN3X"
    "vmT3t2Pty/Yge6Cd0R5iX7Pv4mH2fZyJ3LUf4VF2Hjuv/dAeZ0+wC9gF7cl2PvLCLkhe2zPtEnZh8t6eYy+wS9uf7Z94kf0L/7CX"
    "2CvxUruaXcVeYQNS0+ZJLXudXYtssHV7k72ebLG32jpBtkWa27vsPXZruyXZZ3vkgO2SQ/YRm5Gj9nH7mJ3ITkB62Kfts3YS0oMk"
    "J8nIefuinc6+Yl+1B9tx9k37hp2RDLfv2MPJPfu+PdYeQR7buckTOy/JQ57bU+wi9lR7uv3GfmcXIbPsD/ZHuySZZ3+xS5Fv9mL7"
    "u12e/LL/2JXJb/ufvcpea3OkBhHJersO2WhrZLNdnzS1G5IdtkmaEExsQslee7/divikDeloByQkXe3DJB7pRE7ZiUki0p30ti/Y"
    "KUhqOxXpS/qRNCQ1yUDSkVt2ejKUZCZZyAg7G3lg5yQ5yCN7NHlqP7MLkPzkpf3KnkLe2jFktl2UTCczyCxSgny1S5O5ZD4pR37a"
    "FUhFsoT8tX+R5WQFqU4EUpNIpDZRiErqkY2kAYGkEdlKEGlKmhFCmpMWxCGtyUG7LWlHDpL2JD45aXcmCUlX0o2csZOSU6QXSUxT"
    "knMkLYkl/clAcoUMJoNIHMlhp6fDSFYylI4k98gDMobkoKPpY/KEPCXPSCHynEwjU8lMUoy8JZ/sOeQD+UjKkM+kNP1CvpHvpBJZ"
    "SqqS32QVqUZWkjVkLeFoTVqb1iUGUegmsoU0JttJA7qDWHQn2U32kT1kP6H0EDlAOpCOhNGjJB49Rk6QhPQk6UnOkKQ0CU1Gk9OL"
    "ZADpTfvSq+QauU6GkEE0jt4md0l2MorkIvfJWDKejCMTyEQyiUwmL8gr8pK8Jm9IcfKevCOzyQxalsyiC8gispAsJj/IMvKTVCGV"
    "6F9ShfJ0NQG0BhXoBiJRlWq0Id1M6lNIt5EmdBfZRm26lxDagjrUoz5tS4+QAzSix0l82oUmoKfJWdKb9KDnSU+agvahqeh5mpqm"
    "penoDXKT3CKDaSZ6hwynw2g2+pA8IrlpLjqWjqP56QT6lBakU2ghWpgWpdPoJ1KCzqYl6Vz6lZSh5eh8Wp5WoBXpElqZ/iNVaXVa"
    "ja4jIq1FZVqX1qH1qEF12oiaFNGmFNMdtDltTV26h+6lAW1P29GQdqCH6RHamSaiXWk3eor2Ib3oZXKBXCIXaBoaS/vTAXQgzUAz"
    "0sw0Cx1Cs9LsdAQdSXPSUTQPzebkpeNpATqRPqMxdDJ9SYvQ13Qmneq8o/NIKTqHfqQLaFn6hS6ki+kfspT+pr/oP1rJWUlX07V0"
    "DV1H11PJ2URrO5vpFrqVGk4z2tCxHOTsptghDnX20Tb0ID1EO9GO9Cg9TiPnBO1OT9LT9AxN4pylKWk/epGmcC7R1M4VeoNepRmc"
    "6/Q2vUXv0OzOXTqGPqA5nEf0CR3t5HYm0XzOc5rfmeRMpzFOMVqcFnU+0RLOPPqZLqLfaSnnB/1Gl9GfdDldQVfRvxQ41RzeEZxa"
    "zga6kSqO6mhOfacx3U5NZyfdRW2nJd1PWzotHNfxHc9hTlsnnhM6x2h8J6HTxenqdHPO0Z5OD6eXc42mdFI5l2kaJ9YZ6Nyk6Z2M"
    "zhAni3PduUeHO/fpQ5rTeUzzOPloXqeAM5W+oK/oZOcNfUtfOdOdYs4Mp7hT0pnjlHW+0s9Oeaecs9D5Qys7FZ0qTlWHc2o41Z2a"
    "jujUcWRng1PX0Z0Gjuo2crY6TZ1mzg6nudOK7nFaOXudNs5+J3DaOx2czs5hp5OTwDnhJHa6O0md5M5Zp4/T2znn9HMuOGmddM4A"
    "Z5Az2MnkZHZuOUOdEU5WZ6Rzz8nljHEeO+Odcc4Tp5Az0SnovHCmOEWc9/SNM9v5QGc5pZ15zienjLPIqeAscBY7S5wK7jJnhbPc"
    "WeWsdlY66xzeFdz1Tj2njgsd2dXcxo7hbnOaODsd5DrObqe1085p7h50PPeQE7gdnaPOMSfw4rsnnVPOaSeZc8bp6yRz+zsp3F7e"
    "JSeVe9W55sQ5N5wr7jAnk3vHGered0Y5D5yHziNnrDPByeM+d/K5+d1pzmvnpfPWmekUcT8475yPToxX0i3lfnW+O9+cpc4Pp7z7"
    "y/nj/HWquJz7y13jVHcld7W7ydnobHa2OLrb0N3uWG5jd9f/2t/pUreF67it3DbuAeeI085t7x5yjzuJnE5uQrer291N7CZ1e7rJ"
    "3fPOZeeKk9Lt76Z107kZ3JtORve2c9fJ4o5ws7vZ3OHuXXe0O8Z96OZ1CzvPnAluQXeyW8gt7E5zi7rF3A/uXOeLM9+Z55Z2y7o/"
    "nXLub2exW8mt7P5zfrtrXOCudVa5tdzaruDVc+u6m1zoNnA3u6bbxG3qYnefQ9yWruv67j6XuW3dyA3djm4Ct7PbxU3kdnNPuUnc"
    "i04ir7d71u3rpnYHuGnc9G6sO8i95sa5md303i03qzvKzenmcnO4T53c7jj3iTveneg+dfN5U9yp7ntnujvTfesWd2e5s90y7nx3"
    "rrvQXeQucL+7S9ylblW3mrvCreH+dWu6vAc8xV3v1nc3uFtc1dO8re42F3qmt93F3i53j0u95t5+96DbwfW9eG7ktfOOufG8E+5J"
    "t4fbyz3j9nHPu+fci+4F95J72R3oDnavu1fdIe5Nd5h7243zRrr33QfuMG+k98h97Ob28ngF3OfuC/el+9p9475yS7hFvPduMe+z"
    "W8L75JbySnrf3DLeD/enu8xd7q50q3iVPM4T3bVuDW+1J3myt9Gt6zVy63uNvGbuDnebZ3m218wjXmt3t9fG87wD7mH3iHvUDb0O"
    "XnwvgXfa7eYl9Lp7Sbx+bjKvj5fK6+ul9tJ5F70BXgbvhpvRy+IN9e6499wR3lh3uJfTy+VNcp+5eb0Yd6JXwCvoTfKmeNO9l940"
    "b477zvvofnHneF/dim5Zb6G3wFvkLfb+uP/cyt5yr6q30lvl1fJEb53H+4pXz9O9Bp7hNfa2ets95O1297qOt8tzvVZea2+fx7z2"
    "3mHvuNvZ6+J18rp6ib2k3mmvh5fcS+H19vp5abwLXlovpTfQu+oN9jJ56f2sXmYvm5fdy+GN8sZ4D7xx3kMvvzfem+A99wp5M9zC"
    "3ju3qFfcm+nN8GZ7pb153nxvrlfOq+CV9yp6S7yf3jLvt1fdW+Gtc2t6a7w6Xm1P9Dd4sr/Jq+urfkOvgd/U2+m18HZ4e7yWXnO/"
    "rbffO+gd8A55RzzmH/Ui/5h3wjvlnfR6eon9pP45L5nf34v1Uvqp/CveNW+QN8RL52fwM/q3vbveUP+ed98b7T3yxno5/dz+E2+y"
    "N9V75r3wCvivvTfeW++N/8Er4n/yPnqfvS/eV++7V9Yv5y/1fnnVvD/eXw/4lf21HucL/mp/o1fLV/zN3hZP9w0f+o185Df2m/rY"
    "3+u18qnf0t/r+35bv6MX+se9eP4RP6Hfxe/mn/WS+N39815yP4Xf27/s9fNT+9e9tP5N75Z3zc/kD/Gz+tn8Yf5wP5f/2MvhP/Xy"
    "+OP8fH5B/5WX3y/kT/Vj/Fnee2+mX8yf5X/wS/pl/G/ePL+Cv8Av7y/yK/n/vCp+VX+5v8Jf70n+Gr+2v9av40vBJr+J19A3fctv"
    "4tt+M5/4jr/bd/3WfuC38ff77f0O/mE/gR/f7+of9xP5Pf0zXsKgl9/Hv+T19/v6afxYf4B/1R/o3/DSBVn8m/4db4Sf3R/pj/LH"
    "+Pf9sX5e/4k/3n/qP/ef+VP8wv40v6j/2p/hl/BL+XP8uX5p/4v/w/vqf/eX+BX9pf4y/7dfza/hr/K5oLq/zhcDza/n1/flYLO/"
    "1dcCI9ju7/BRYAUt/D2+5+/z2/lecMA/5LOgk3/U7+zHD+IFJ/0EwWn/rH/GP+f3DFIEF/zLfupgkD/Yv+LH+Tf8zH7G4LafLcgS"
    "3PVH+w/8h/5j/5E/0Z/gT/In+y/9F/50/5Vf3H/rv/OLBu/9j/4nf77/2S8dLPbLBL/8H/5Pv1JQMVjp//Nr+iuC6oEQrPc3+EpQ"
    "O6gTbPH1YJvfINjpm0HjYJdvBzRoHrQI3OCg7wdB0NEPg2P+Cf9Q0MM/5R8PEgdJgqTBeb9P0Cu46KcK+gaxQdrgup8+uOUPDjIF"
    "mYM7/j0/a5AzyB7kCHIFD4K8QZ4gX5A/yMOeBS+CwkGhYFow2y8WFA9mBiWCOcFC/5tfNigXlA8qBH/9P37loGoAgipBzYAPagU1"
    "go3+mqBuUD/YEMCgYbA9aBTgoGmwI2gZkMAJWgWtg73B/sBj7YIoCFinoEuQKOgWMNY9SBb0CJIHKYPewSU/TdAvuBAMCC4HGYJB"
    "QVwwJLgZDAvuBiOC28G9IHcwNhgdPArGBxOCAkHB4HkQE0wNpgdFgjdByWBWMDsoFXwM5gWfggXBouB7sDBYFvwIKrBqwe/gb7Ay"
    "WBusCtYFarA+qBeoTGabAo1tCZoE24JmgcV2BTbbHewL2gRtA5e1DzoER4OOQecgYvHYiSAhS8DOBmeC08H5ICnrH1wKrgQXg4HB"
    "teB6kIYNDTKwW0EmdicYGYwK7gdjghxsXJCL5WYTg8nBpGBKUIC9DgqyGcHbYBp7H3wI5gYl2fzga/Al+BYsCX4Gi4OlwfKgElsd"
    "/GKAVWPVmcBqMokpbGOwOdgaNGAGg6wx2xkgtidoxghrwVqyA4HPDgaHgyPBseAQ6xrEZyeDKOzKErPu7FxwiiVjKVkKlpqlYv1Z"
    "LLsa3AjSs8EsMxvGhgfZWHaWlT0MHgc52dPgSZAzzM/ysZdBDJvMCrF3QRFWlBVnxVgJ9jmYw+azr6w0K8vKswWsIqvC/gQc+xdU"
    "ZTxbxURWi9VmdVg9prNNrD5ryEy2hTVhmG1nzRllDtvDWrM2rC1rx9qzkHVkR1gndipIxE6ybqwn68VOs7OsLzvH+rELLC0byAaw"
    "Qew6y8iGsJtsKBvBhrORLGs4ho1m2cOxbAIbzyax5+xVMIVNZYXZTDadzWBv2WxWipVh89hcVo4tYr+CxewHq8yWst/sL/vD/rHV"
    "rGpYl61jQriBbWRr2WamhlvZNgbDHWwn2812sVZsL3PCfcwND7DDrAPrzMKwCzvOTrAkrAfrGiZnvdl5ljS8yNKxS+wyu8Kusmss"
    "jg0MM4bDwgzhXTaKZQtvsxzhOPaY5WW5w4nsKXvGXrACYcHwNSsexoSz2HtWNPzISoRf2Ge2kJUOy4ZL2E+2jK1gy9lKVoOtYVwo"
    "hqtCKVzP6oRaKIeNmB4aYVPWKDRDFOLQDklIw/2sbeiFrUM/bBceZcdYxzBeeCxMEJ5h3cLEYfKwD0sZpgh7h33C1GGaMG04IIwN"
    "s7C48A67xbKEQ8P7bHj4kN0Ln7DR4YMwT5Q3HB++Yi/Z5LBQ+IYVDt+xD6xY+ImVDOeGc8Iy4Tf2JSwXVgorhBXDJeGvEITVw2oh"
    "H9YMa4dKWC9cG9YNN4Zy1DjcEjYMt4fNwqZhi3BH2DJsFR5kNArCg2GH0IuOhIfDhOHRMFHYPUwSng57hr3C7lHfsF+YKuwfpgtv"
    "sPTh1XBwmCkcEmYO77ER4QM2MnzE7odjwzxhvnBcmD+cGBYJp4RTw0nhtHB6OCMsHM0KZ4elwnnh/PA7WxAuCheG5aOyUeWwSrg8"
    "XBH+DVeHNUIQ8VGtcH0oRfXDBqEaWWGDaFvYPNwaWtHucFe4J8RRm9CJDoRu1D7sFHYO44ddwhPhyfB42CNMGJ0Jk4XJonNhryhV"
    "dDG8FKaJBoXXwhvhgOhmeCu8Hd4N74SjwuHRmDBn9DB8FD4J80b5oqdh/uh1WCB6Fb4JZ4bvw2LRx7BI9Cn8HJYPv4bfwsXh0nBZ"
    "+CP8E/4OuWhl+C+sGq0Lq0ditCFUovrR5lCLmoR6ZEY7QxTByI6aRiTaH+4LWdQ6OhT6UduoQxQviqJOUecoUdQtShwljZJEPaLz"
    "YYqod5QyuhL2j2Kj6+HAKEOUOcoYZYqGRFmiXOGIaGT0OMwV5Y4mhM/C5+GL8GVYKHobFoxioqLRu/BDWDwqGZWISkWlozLR97Bc"
    "9DOsEFWKKkaVoypRtWhNWCsSohpR7ahmtCmsE9WN6kVG1DBqFDWOmkTNo71hs6hV1DJqEbGwXdQm+h9OUceofZQgih91ibpGp8Kz"
    "Yc/oQpg86htdDvtEqaN+UboobZQ+GhzFRYOioVHWKFs0LMoe5YhGR6OiMdHYaFz0H0fqCco="
)

N_CORES = 8
B, C, H, W = 512, 9, 84, 84
NPOS = H * W                      # 7056
ROWS = (B // N_CORES) * C         # 576 rows per core
D = 18                            # rows per (partition, index)
SETS = 2                          # 576 = 2 * 16 * 18
NSLICE = 16
SL = NPOS // NSLICE               # 441 positions per slice
NI = 224                          # indices per sub-gather (multiple of 16)
ILENS = [224, SL - 224]           # sub-gather coverage (224, 217)
# slice -> (round, group) assignment, sorted descending by tight window so
# each round's DMA-in decomposes into partition-prefix "staircase" transfers
SLICES = [[2, 7, 5, 6, 11, 14, 9, 15], [4, 8, 3, 13, 10, 12, 1, 0]]
SLICE_LO = [0, 316, 576, 1078, 1477, 2060, 2424, 2874,
            3287, 3821, 4248, 4684, 5132, 5529, 6007, 6450]
SLICE_WL = [597, 734, 947, 857, 927, 883, 822, 906,
            888, 752, 770, 772, 759, 811, 766, 606]
WLR = [[SLICE_WL[t] for t in SLICES[r]] for r in range(2)]   # desc per round
NER = [max(w) for w in WLR]       # [947, 927] gather num_elems per round


def _perm() -> np.ndarray:
    raw = zlib.decompress(base64.b64decode(_PERM_B64))
    assert zlib.crc32(raw) == 3102470381
    return np.frombuffer(raw, dtype="<i2").astype(np.int32)


def _idx_arrays() -> list[np.ndarray]:
    """Wrapped int16 index tiles, one per (round, sub): [128, NI//16]."""
    perm = _perm()
    tiles = []
    for r in range(2):
        for sub in range(2):
            arr = np.zeros((128, NI // 16), dtype=np.int16)
            for g in range(8):
                t = SLICES[r][g]
                ilen = ILENS[sub]
                vals = perm[t * SL + 224 * sub : t * SL + 224 * sub + ilen] - SLICE_LO[t]
                assert vals.min() >= 0 and vals.max() < SLICE_WL[t] <= NER[r]
                vals = np.concatenate([vals, np.zeros(NI - ilen, np.int64)])
                arr[16 * g : 16 * (g + 1)] = (
                    vals.reshape(NI // 16, 16).T.astype(np.int16)
                )
            tiles.append(arr)
    return tiles


def _encode(shard: np.ndarray) -> list[np.ndarray]:
    """shard (576, 7056) -> [xa (2, 128, 947*18), xb (2, 128, 927*18)]:
    x[r][s, 16g+q, (j, rslot)] = shard[288s + 18q + rslot, SLICE_LO[t] + j]
    for j < wl(t), t = SLICES[r][g]; zeros beyond wl (never transferred)."""
    out = []
    rows = shard.reshape(SETS, 16, D, NPOS)      # s, q, rslot, pos
    for r in range(2):
        ne = NER[r]
        buf = np.zeros((SETS, 128, ne, D), dtype=np.float32)
        for g in range(8):
            t = SLICES[r][g]
            lo, wl = SLICE_LO[t], SLICE_WL[t]
            win = rows[:, :, :, lo : lo + wl]            # s, q, rslot, j
            buf[:, 16 * g : 16 * (g + 1), :wl, :] = win.transpose(0, 1, 3, 2)
        out.append(np.ascontiguousarray(buf.reshape(SETS, 128, ne * D)))
    return out


def _decode(od: np.ndarray) -> np.ndarray:
    """od (8, 128, NI, D) blocks (s*4 + r*2 + sub) -> (576, 7056)."""
    res = np.empty((SETS, 16, D, NPOS), dtype=np.float32)   # s, q, rslot, pos
    for s in range(SETS):
        for r in range(2):
            for sub in range(2):
                blk = od[s * 4 + r * 2 + sub].reshape(8, 16, NI, D)
                ilen = ILENS[sub]
                for g in range(8):
                    t = SLICES[r][g]
                    c0 = t * SL + 224 * sub
                    res[s, :, :, c0 : c0 + ilen] = blk[g, :, :ilen, :].transpose(
                        0, 2, 1
                    )
    return res.reshape(ROWS, NPOS)


_CACHE = {}


def _build_program():
    from concourse import bacc, mybir
    from concourse.tile import TileContext

    nc = bacc.Bacc(
        "TRN2", target_bir_lowering=False, debug=False, num_devices=N_CORES
    )
    f32 = mybir.dt.float32
    i16 = mybir.dt.int16

    x_ds = [
        nc.dram_tensor(f"x{r}", [SETS, 128, NER[r] * D], f32, kind="ExternalInput")
        for r in range(2)
    ]
    idx_ds = [
        nc.dram_tensor(f"idx{j}", [128, NI // 16], i16, kind="ExternalInput")
        for j in range(4)
    ]
    o_d = nc.dram_tensor("o", [8, 128, NI * D], f32, kind="ExternalOutput")

    with TileContext(nc) as tc:
        with tc.tile_pool(name="sb", bufs=1) as pool:
            I = [
                pool.tile([128, NER[r] * D], f32, tag=f"I{r}", name=f"I{r}")
                for r in range(2)
            ]
            G = [
                pool.tile([128, NI * D], f32, tag=f"G{j}", name=f"G{j}")
                for j in range(3)
            ]
            IX = [
                pool.tile([128, NI // 16], i16, tag=f"IX{j}", name=f"IX{j}")
                for j in range(4)
            ]
            for j in range(4):
                nc.sync.dma_start(out=IX[j][:, :], in_=idx_ds[j].ap())
            # staircase loads leave per-group window tails untouched; they
            # are never gathered (idx < wl) but must be initialized once
            for r in range(2):
                wmin = WLR[r][7]
                nc.vector.memset(I[r][:, wmin * D :], 0.0)

            gk = 0
            for s in range(SETS):
                for r in range(2):
                    # staircase DMA-in: prefix of 16*(k+1) partitions covers
                    # j-range [wl[k+1], wl[k]) (wl descending per round)
                    wl = WLR[r]
                    j0 = 0
                    for k in range(7, -1, -1):
                        j1 = wl[k]
                        if j1 > j0:
                            nc.sync.dma_start(
                                out=I[r][: 16 * (k + 1), j0 * D : j1 * D],
                                in_=x_ds[r].ap()[s][: 16 * (k + 1), j0 * D : j1 * D],
                            )
                        j0 = j1
                    for sub in range(2):
                        gt = G[gk % 3]
                        gk += 1
                        nc.gpsimd.ap_gather(
                            gt[:, :],
                            I[r][:, :],
                            IX[r * 2 + sub][:, :],
                            channels=128,
                            num_elems=NER[r],
                            d=D,
                            num_idxs=NI,
                        )
                        nc.scalar.dma_start(
                            out=o_d.ap()[s * 4 + r * 2 + sub],
                            in_=gt[:, :],
                        )
    nc.compile()
    return nc


def _get_program():
    if "nc" not in _CACHE:
        _CACHE["nc"] = _build_program()
    return _CACHE["nc"]


def kernel(x, _trace=False):
    from concourse import bass_utils

    x = np.asarray(x, dtype=np.float32)
    assert x.shape == (B, C, H, W), x.shape
    shards = np.ascontiguousarray(x).reshape(N_CORES, ROWS, NPOS)
    idx = _idx_arrays()

    nc = _get_program()
    in_maps = []
    for i in range(N_CORES):
        xa, xb = _encode(shards[i])
        in_maps.append(
            {"x0": xa, "x1": xb, **{f"idx{j}": idx[j] for j in range(4)}}
        )
    res = bass_utils.run_bass_kernel_spmd(
        nc, in_maps, core_ids=list(range(N_CORES)), trace=_trace
    )
    _CACHE["last_results"] = res
    out = np.stack(
        [_decode(res.results[i]["o"].reshape(8, 128, NI, D)) for i in range(N_CORES)]
    )
    return out.reshape(B, C, H, W)

